# revision 51
# baseline (speedup 1.0000x reference)
"""GCN diag-encoder (2-layer SpMM) on 8 Trainium2 NeuronCores.

Strategy: the sparse adjacency (640K edges over 10K nodes, ~0.64% dense) is
materialized as a dense A^T matrix on the host; each per-layer
  out[dst] = sum_e vals[e] * x[src[e]]        (segment-sum SpMM)
becomes dense TensorEngine matmuls.  Each core owns a 1250-wide dst slice of
A^T (padded to 1280, uint8-quantized per dst column) and streams A^T k-tiles
from HBM with an inline u8->f16 cast in the DMA, in variable-size k-tile
groups (small first/last groups shorten the pipeline ramp and tail).

Layer 1 runs A-stationary — matmul(out=psum[dst,feat], lhsT=AT_tile[src,dst],
rhs=x_tile[src,feat]) — so the layer-1 output is already node-major: the
eviction is a fused tanh+dequant-scale pass on the scalar engine (scale is
per dst node = per partition) straight into the AllGather bounce.  PSUM
accumulation groups are per 2KiB bank while layer 1 writes four 512B ranges
per bank, so each bank is seeded by one full-width start=True zero matmul.
Layer 2 (PE-bound) runs X-stationary — matmul(out=psum[feat,dst],
lhsT=x1_tile[src,feat], rhs=AT_tile[src,dst]); its dequant scale (per dst =
per free element) and the final transpose are applied on the host.

Src nodes use a padded rank-block ordering (rank r owns slots
r*1280..r*1280+1279) so layer 2's AllGathered activations line up with the
SAME A arrangement layer 1 uses — the first RESG k-tile groups of A stay
resident in SBUF for layer 2, and layer 2 interleaves resident/streamed
groups so PE starts on the earliest-arriving x1 chunks while the remaining
A-stream DMAs land.  W0 is folded into x on the host; W1 is skipped on
device when it is all-ones (torch init), else applied via a broadcast
multiply.
"""

import numpy as np
import ml_dtypes

N = 10000          # nodes
D = 128            # feature dim
NCORES = 8
S = 1250           # dst nodes per core
SP = 1280          # padded dst per core (10 tiles of 128)
KT = 80            # contraction k-tiles (padded src rows = 10240)
NPAD = KT * 128    # 10240
GSIZES = (8, 8, 8, 8, 8, 8, 8, 8, 8, 8)   # k-tiles per group
RESG = 6           # leading groups kept resident in SBUF for layer 2
BF16 = ml_dtypes.bfloat16

_PROG_CACHE = {}


def _groups():
    out = []
    k0 = 0
    for sz in GSIZES:
        out.append((k0, k0 + sz))
        k0 += sz
    assert k0 == KT
    return out


def _build_program(nocc=False, skip=(), u8=True, resg=RESG, abufs=2,
                   w1_ones=True, l2order="streamfirst", gsizes=GSIZES):
    import concourse.bacc as bacc
    import concourse.mybir as mybir
    from concourse import tile

    f32 = mybir.dt.float32
    f16 = mybir.dt.float16
    adt = mybir.dt.uint8 if u8 else f16
    grps = []
    _k0 = 0
    for _sz in gsizes:
        grps.append((_k0, _k0 + _sz))
        _k0 += _sz
    assert _k0 == KT
    maxg = max(k1 - k0 for k0, k1 in grps)

    nc = bacc.Bacc(
        "TRN2",
        target_bir_lowering=False,
        debug=False,
        enable_asserts=False,
        num_devices=1 if nocc else NCORES,
    )

    a = nc.dram_tensor("a", [KT, 128, SP], adt, kind="ExternalInput").ap()
    # f16 copy of the streamed (non-resident) k-range: layer 2 refetches it
    # on the sync HWDGE ring, FIFO-ordered behind the critical x1 loads
    ks0 = grps[resg][0] if resg < len(grps) else KT
    af = nc.dram_tensor(
        "af", [max(KT - ks0, 1), 128, SP], f16, kind="ExternalInput"
    ).ap()
    x0 = nc.dram_tensor("x0", [128, NPAD], f16, kind="ExternalInput").ap()
    # per-dst-node dequant scales, [slot p, tile t] layout
    csc = nc.dram_tensor("csc", [128, 10], f32, kind="ExternalInput").ap()
    # broadcast W1 row (only read when not w1_ones)
    w1b = nc.dram_tensor("w1b", [128, 128], f16, kind="ExternalInput").ap()
    out = nc.dram_tensor("out", [128, SP], f32, kind="ExternalOutput").ap()

    with tile.TileContext(nc) as tc:
        with (
            tc.tile_pool(name="xp", bufs=1) as xp,
            tc.tile_pool(name="ab", bufs=abufs) as apool,
            tc.tile_pool(name="res", bufs=1) as rpool,
            tc.tile_pool(name="ev", bufs=1) as ev,
            tc.tile_pool(name="ps", bufs=1, space="PSUM") as ps,
            tc.tile_pool(name="dr", bufs=1, space="DRAM") as dr,
        ):
            # x0 is dead once layer 1 finishes; share one slot for both
            x0s = xp.tile([128, NPAD], f16, tag="xs")
            x1s = xp.tile([128, NPAD], f16, tag="xs")
            cscs = xp.tile([128, 10], f32, tag="cscs")
            w1s = xp.tile([128, 128], f16, tag="w1s")
            zl = xp.tile([128, 512], f16, tag="zl")
            warm = xp.tile([128, 1], f32, tag="warm")
            nc.scalar.dma_start(cscs[:], csc)
            if not w1_ones:
                nc.scalar.dma_start(w1s[:], w1b)
            nc.vector.memset(zl[:], 0.0)
            # pre-load the ACT tanh table so the layer-1 eviction doesn't
            # pay the table load on the critical path
            nc.scalar.activation(
                warm[:], zl[:, 0:1], mybir.ActivationFunctionType.Tanh
            )

            agin = dr.tile([128, SP], f16)
            agout = dr.tile([NCORES * 128, SP], f16, addr_space="Shared")

            res_tiles = {}

            def fetch_group(gi, halves=1, via_f16=False):
                """DMA group gi of A into an SBUF tile (f16, cast if u8)."""
                k0, k1 = grps[gi]
                nk = k1 - k0
                if gi < resg:
                    ab = rpool.tile([128, nk * SP], f16, tag=f"res{gi}")
                    res_tiles[gi] = ab
                else:
                    ab = apool.tile([128, maxg * SP], f16, tag="ab")
                if "adma" in skip:
                    nc.gpsimd.dma_start(ab[:, 0:8], a[k0][:, 0:8])
                    return ab
                bounds = [k0 + (nk * h) // halves for h in range(halves + 1)]
                for b0, b1 in zip(bounds, bounds[1:]):
                    if b0 == b1:
                        continue
                    dst = ab[:, (b0 - k0) * SP:(b1 - k0) * SP].rearrange(
                        "p (k j) -> p k j", k=b1 - b0
                    )
                    if via_f16:
                        src = af[b0 - ks0:b1 - ks0].rearrange("k p j -> p k j")
                        nc.sync.dma_start(dst, src)
                    elif u8:
                        src = a[b0:b1].rearrange("k p j -> p k j")
                        nc.gpsimd.dma_start(dst, src)
                    else:
                        src = a[b0:b1].rearrange("k p j -> p k j")
                        nc.sync.dma_start(dst, src)
                return ab

            # ---- layer 1 (A-stationary; psum is [dst slot, feat]) ----
            psum1 = ps.tile([128, SP], f32, tag="acc1")
            for c0, cn in ((0, 512), (512, 512), (1024, 256)):
                nc.tensor.matmul(
                    psum1[:, c0:c0 + cn], zl[:, 0:128], zl[:, 0:cn],
                    start=True, stop=False,
                )
            for gi, (k0, k1) in enumerate(grps):
                nc.scalar.dma_start(
                    x0s[:, k0 * 128:k1 * 128], x0[:, k0 * 128:k1 * 128]
                )
                ab = fetch_group(gi, halves=2 if gi == 0 else 1)
                if gi < len(grps) - 1:
                    for k in range(k0, k1):
                        kk = k - k0
                        rhs = x0s[:, k * 128:(k + 1) * 128]
                        for t in range(10):
                            nc.tensor.matmul(
                                psum1[:, t * 128:(t + 1) * 128],
                                ab[:, kk * SP + t * 128:
                                   kk * SP + (t + 1) * 128],
                                rhs,
                                start=False, stop=False,
                            )
                else:
                    # final group t-outer: each dst range finishes early so
                    # the tanh eviction overlaps the remaining matmuls
                    for t in range(10):
                        for k in range(k0, k1):
                            kk = k - k0
                            nc.tensor.matmul(
                                psum1[:, t * 128:(t + 1) * 128],
                                ab[:, kk * SP + t * 128:
                                   kk * SP + (t + 1) * 128],
                                x0s[:, k * 128:(k + 1) * 128],
                                start=False,
                                stop=(k == KT - 1 and t in (3, 7, 9)),
                            )

            # evict layer 1: x1 = tanh(cs_dst * psum1) [* W1] on ACT, chunked
            # agin DMA so the AllGather input lands as soon as possible.
            agin_sb = ev.tile([128, SP], f16, tag="agin")
            for t in range(10):
                c0, c1 = t * 128, (t + 1) * 128
                nc.scalar.activation(
                    agin_sb[:, c0:c1], psum1[:, c0:c1],
                    mybir.ActivationFunctionType.Tanh,
                    scale=cscs[:, t:t + 1],
                )
                if not w1_ones:
                    nc.vector.tensor_mul(
                        agin_sb[:, c0:c1], agin_sb[:, c0:c1], w1s[:]
                    )
                nc.scalar.dma_start(agin[:, c0:c1], agin_sb[:, c0:c1])

            residents_pre = list(range(resg))
            streams_pre = list(range(resg, len(grps)))
            if l2order == "streamfirst":
                _order_preview = streams_pre[:abufs] + residents_pre + streams_pre[abufs:]
            elif l2order == "weave":
                _order_preview = []
                for i in range(2):
                    if i < len(streams_pre):
                        _order_preview.append(streams_pre[i])
                    if i < len(residents_pre):
                        _order_preview.append(residents_pre[i])
                _order_preview += residents_pre[2:] + streams_pre[2:]
            else:
                _order_preview = None

            if nocc:
                nc.scalar.dma_start(agout[0:128, :], agin[:])
            else:
                nc.gpsimd.collective_compute(
                    "AllGather",
                    mybir.AluOpType.bypass,
                    replica_groups=[list(range(NCORES))],
                    ins=[agin.opt()],
                    outs=[agout.opt()],
                )
            # agout rank blocks laid side by side in the free dim are exactly
            # layer-2's lhsT tiles in the same padded rank-block order A uses.
            rank_order = []
            for gi in _order_preview:
                k0, k1 = grps[gi]
                for r in ((k0 * 128) // SP, ((k1 * 128) - 1) // SP):
                    if r not in rank_order:
                        rank_order.append(r)
            for r in range(NCORES):
                if r not in rank_order:
                    rank_order.append(r)
            for r in rank_order:
                nc.sync.dma_start(
                    x1s[:, r * SP:(r + 1) * SP],
                    agout[r * 128:(r + 1) * 128, :],
                )

            # ---- layer 2 (X-stationary; psum is [feat, dst]) ----
            # Interleave: residents first (rank-0 x1 chunk arrives first),
            # streamed groups spread out so their DMAs pipeline through the
            # abufs slots while PE chews residents.
            psum2 = ps.tile([128, SP], f32, tag="acc2")
            residents = residents_pre
            streams = streams_pre
            if l2order == "streamfirst":
                order = streams[:abufs] + residents + streams[abufs:]
            elif l2order == "weave":
                # s0 r0 s1 r1 r2 ... then remaining streams at the tail
                order = []
                for i in range(2):
                    if i < len(streams):
                        order.append(streams[i])
                    if i < len(residents):
                        order.append(residents[i])
                order += residents[2:] + streams[2:]
            else:
                order = []
                ri, si = 0, 0
                pattern = [0, 0, 1, 0, 1, 0, 1, 0, 1, 0, 1, 0]  # 1 = stream
                for p in pattern[:len(grps)]:
                    if p and si < len(streams):
                        order.append(streams[si]); si += 1
                    elif ri < len(residents):
                        order.append(residents[ri]); ri += 1
                order += residents[ri:] + streams[si:]
                if order[-1] in streams:
                    for i in range(len(order) - 2, -1, -1):
                        if order[i] in residents:
                            order.append(order.pop(i))
                            break

            ob = ev.tile([128, SP], f32, tag="ob")
            first = True
            for oi, gi in enumerate(order):
                k0, k1 = grps[gi]
                ab = (res_tiles[gi] if gi < resg
                      else fetch_group(gi, via_f16=True))
                last_grp = oi == len(order) - 1
                if not last_grp:
                    for k in range(k0, k1):
                        kk = k - k0
                        lhsT = x1s[:, k * 128:(k + 1) * 128]
                        for c0, cn in ((0, 512), (512, 512), (1024, 256)):
                            nc.tensor.matmul(
                                psum2[:, c0:c0 + cn],
                                lhsT,
                                ab[:, kk * SP + c0: kk * SP + c0 + cn],
                                start=first, stop=False,
                            )
                        first = False
                else:
                    # final group: bank-outer so each psum2 bank completes
                    # (stop=True) early and its eviction overlaps the rest
                    for c0, cn in ((0, 512), (512, 512), (1024, 256)):
                        for k in range(k0, k1):
                            kk = k - k0
                            nc.tensor.matmul(
                                psum2[:, c0:c0 + cn],
                                x1s[:, k * 128:(k + 1) * 128],
                                ab[:, kk * SP + c0: kk * SP + c0 + cn],
                                start=False, stop=(k == k1 - 1),
                            )
                        nc.vector.tensor_copy(
                            ob[:, c0:c0 + cn], psum2[:, c0:c0 + cn]
                        )
                        nc.sync.dma_start(
                            out[:, c0:c0 + cn], ob[:, c0:c0 + cn]
                        )

    nc.compile()
    return nc


def get_program(nocc=False, skip=(), u8=True, resg=RESG, abufs=2,
                w1_ones=True, l2order="streamfirst", gsizes=GSIZES):
    key = ("nc", nocc, tuple(skip), u8, resg, abufs, w1_ones, l2order,
           tuple(gsizes))
    if key not in _PROG_CACHE:
        _PROG_CACHE[key] = _build_program(nocc, skip, u8, resg, abufs,
                                          w1_ones, l2order, gsizes)
    return _PROG_CACHE[key]


def _node_perm():
    """Padded rank-block src ordering: slot i <-> (rank r = i//1280,
    local q = i%1280); global node r*1250+q for q<1250, else pad."""
    i2 = np.arange(NPAD)
    r2 = i2 // SP
    loc = i2 % SP
    node = r2 * S + loc
    valid = loc < S
    return np.where(valid, node, 0), valid


def build_in_maps(x, src, dst, vals, W, u8=True):
    """Host-side prep: dense A^T shard (u8 per-column quantized) + x0."""
    import scipy.sparse as sp

    x = np.asarray(x, np.float32)
    src = np.asarray(src, np.int64)
    dst = np.asarray(dst, np.int64)
    vals = np.asarray(vals, np.float32)
    W = np.asarray(W, np.float32)

    # A[dst, src] = sum of vals  ->  we build AT[src, dst]
    AT = sp.coo_matrix((vals, (src, dst)), shape=(N, N)).toarray()

    node2, valid2 = _node_perm()

    xw = x * W[0][None, :]
    x0p = np.zeros((NPAD, D), np.float32)
    x0p[valid2] = xw[node2[valid2]]
    x0h = np.ascontiguousarray(
        x0p.reshape(KT, 128, D).transpose(1, 0, 2).reshape(128, KT * D)
    ).astype(np.float16)

    w1brow = np.ascontiguousarray(
        np.tile(W[1][None, :], (128, 1))
    ).astype(np.float16)

    in_maps = []
    steps = []
    for c in range(NCORES):
        ATc = AT[:, c * S:(c + 1) * S]  # [N, S] float32
        colmax = np.maximum(ATc.max(axis=0), 1e-9)
        step = colmax / 255.0
        if u8:
            Aq = np.clip(np.rint(ATc * (1.0 / step)[None, :]), 0, 255).astype(
                np.uint8
            )
        else:
            Aq = (ATc * (1.0 / step)[None, :]).astype(np.float16)
        Ap = np.zeros((NPAD, SP), Aq.dtype)
        Ap[valid2, :S] = Aq[node2[valid2]]
        step_pad = np.zeros(SP, np.float32)
        step_pad[:S] = step
        steps.append(step_pad)
        # csc[p, t] = dequant scale of dst slot t*128+p
        csc_tile = np.ascontiguousarray(step_pad.reshape(10, 128).T).astype(
            np.float32
        )
        a3 = np.ascontiguousarray(Ap.reshape(KT, 128, SP))
        ks0 = sum(GSIZES[:RESG])
        in_maps.append(
            {
                "a": a3,
                "af": np.ascontiguousarray(a3[ks0:].astype(np.float16)),
                "x0": x0h,
                "csc": csc_tile,
                "w1b": w1brow,
            }
        )
    return in_maps, steps


def assemble_output(results, steps):
    outs = []
    for c in range(NCORES):
        ot = np.asarray(results[c]["out"], np.float32)  # [128, SP] feat-major
        ot = ot * steps[c][None, :]  # per-dst dequant (layer-2)
        outs.append(ot[:, :S].T)
    return np.ascontiguousarray(np.concatenate(outs, axis=0))


def kernel(x, src, dst, vals, W):
    from concourse import bass_utils

    w1_ones = bool(np.all(np.asarray(W)[1] == 1.0))
    nc = get_program(w1_ones=w1_ones)
    in_maps, steps = build_in_maps(x, src, dst, vals, W)
    # The axon terminal can wedge when a different program was loaded
    # earlier in its lifetime; after the crash the terminal restarts and a
    # retry succeeds.  Back off progressively to ride out the restart.
    import time as _time

    last_err = None
    for sleep_s in (10.0, 30.0, 60.0, 0.0):
        try:
            res = bass_utils.run_bass_kernel_spmd(
                nc, in_maps, core_ids=list(range(NCORES))
            )
            return assemble_output(res.results, steps)
        except Exception as e:  # noqa: BLE001
            last_err = e
            _time.sleep(sleep_s)
    raise last_err
